# revision 1
# baseline (speedup 1.0000x reference)
"""BSRNN mask-generator kernel for 8 Trainium2 NeuronCores.

Strategy (data-parallel over batch, one batch element per core):
  - gLN is folded into the 1x1 conv:  y = istd*(Wg @ x) + e  where
    Wg = conv_w * gamma (host-folded), e = conv_b + W@beta - istd*mean*(W@gamma).
    istd/e are computed on-chip from per-band statistics (bn_stats/bn_aggr +
    PE-transpose + reduce), then applied as per-partition scale/bias inside the
    PSUM->SBUF relu activation.
  - Bands are packed into 17 "chunks" of <=128 conv output rows so every
    engine op runs with (close to) all 128 partitions active. Per-band matmuls
    use zero-padded [128,128] stationary tiles accumulated into one PSUM tile
    (matmul outputs cannot start at a nonzero partition).
  - Chunk row order is (gate g, band, r, s, j) so the sigmoid/gating halves are
    contiguous partition ranges.
  - Complex masking: U = m*CTXA, V = m*CTXB with host-baked signed/replicated
    context rows, then a 0/1 selector matmul collapses the r-dimension:
    est_real = Ssel^T U, est_imag = Ssel^T V.
"""
import sys
for p in ('/opt/trn_rl_repo', '/root/.axon_site/_ro/trn_rl_repo'):
    if p not in sys.path:
        sys.path.insert(0, p)
import numpy as np

WIN, SR, N_SRC, C, T, B = 512, 16000, 2, 128, 1000, 8
EPS = 1e-8
BAND_WIDTH = [3] * 10 + [8] * 12 + [16] * 8 + [3]
N_BANDS = 31
ENC = 257
HALF = T // 2  # 500

# chunks of whole bands, <=128 conv rows (8*bw per band)
CHUNKS = [list(range(0, 5)), list(range(5, 10)),
          [10, 11], [12, 13], [14, 15], [16, 17], [18, 19], [20, 21],
          [22], [23], [24], [25], [26], [27], [28], [29], [30]]
PAIRS = [(0, 1), (2, 3), (4, 5), (6, 7), (8, 9), (10, 11), (12, 13),
         (14, 15), (16,)]
N_CHUNKS = len(CHUNKS)
N_PAIRS = len(PAIRS)

BAND_OFF = np.concatenate([[0], np.cumsum(BAND_WIDTH)]).astype(int)  # freq offsets
CHUNK_OF_BAND = {}
for ci, bands in enumerate(CHUNKS):
    for b in bands:
        CHUNK_OF_BAND[b] = ci
# first band index of each chunk (bands are chunk-contiguous)
CHUNK_BOFF = [bands[0] for bands in CHUNKS]


def _chunk_geometry():
    """Per chunk: band list, g0 row offsets, m-row and z-row maps."""
    geo = []
    for bands in CHUNKS:
        g0off, acc = [], 0
        for b in bands:
            g0off.append(acc)
            acc += 4 * BAND_WIDTH[b]
        geo.append({"bands": bands, "g0off": g0off, "g0rows": acc})
    return geo


GEO = _chunk_geometry()
# est (output) rows per chunk: 2*bw*nb, ordered (s, band, j)
EST_ROWS = [2 * sum(BAND_WIDTH[b] for b in g["bands"]) for g in GEO]
PAIR_EST_ROWS = [sum(EST_ROWS[c] for c in p) for p in PAIRS]
MBASE = {}  # chunk -> base row in the pair's m tile
ESTOFF = {}  # chunk -> base row in the pair's est tile
PAIR_OF_CHUNK = {}
for pi, p in enumerate(PAIRS):
    off = 0
    for k, c in enumerate(p):
        PAIR_OF_CHUNK[c] = pi
        MBASE[c] = 64 * k
        ESTOFF[c] = off
        off += EST_ROWS[c]

# processing order: small/fast chunks first so the pipeline fills quickly
CHUNK_ORDER = [8, 9, 10, 11, 12, 13, 14, 15, 2, 3, 4, 5, 6, 7, 0, 1, 16]
# stats batches (single batch measured fastest)
BATCH_CHUNKS = [CHUNK_ORDER]
# band -> position in batch-processing order (glob row layout)
BPOS = {}
for _ci in CHUNK_ORDER:
    for _b in CHUNKS[_ci]:
        BPOS[_b] = len(BPOS)
BATCH_BANDS = [[b for c in bc for b in CHUNKS[c]] for bc in BATCH_CHUNKS]
BATCH_BSTART = [0] + list(np.cumsum([len(b) for b in BATCH_BANDS]))[:-1]

F32R_BANDS = True  # band matmuls in fp32r (4x faster PE, ~1e-4 rel err)
F32R_EST = True    # selector matmuls in fp32r (rounds U/V at ~5e-4 ulp)

_PROGRAM = None   # (nc,) cache
_CONSTS = None    # host-baked shared tensors cache


def _bake_consts(conv_w, conv_b, gamma, beta):
    """Shared (batch-independent) constant tensors."""
    f32 = np.float32
    # folded weights per band in chunk-row order, zero-padded to [128,128]
    wt = np.zeros((N_BANDS, C, 128), f32)          # [band, c(K), chunk_row(M)]
    wb = np.zeros((128, N_CHUNKS), f32)            # conv_b + W@beta per chunk row
    wg = np.zeros((128, N_CHUNKS), f32)            # W@gamma per chunk row
    rowsel = np.zeros((N_BANDS, 128), f32)         # band -> its chunk rows
    for ci, g in enumerate(GEO):
        for k, b in enumerate(g["bands"]):
            bw = BAND_WIDTH[b]
            Wb = conv_w[b]                          # [oc(128), c(128)] (oc used: 8bw)
            Wgam = Wb @ gamma[b]                    # [128]
            Wbet = conv_b[b] + Wb @ beta[b]         # [128]
            Wfold = Wb * gamma[b][None, :]          # [oc, c]
            for gg in range(2):
                for r in range(2):
                    for s in range(2):
                        ocs = (((gg * 2 + r) * 2 + s) * bw) + np.arange(bw)
                        zrows = (gg * 64 + g["g0off"][k] + r * 2 * bw + s * bw
                                 + np.arange(bw))
                        wt[b, :, zrows] = Wfold[ocs, :]
                        wb[zrows, ci] = Wbet[ocs]
                        wg[zrows, ci] = Wgam[ocs]
                        rowsel[BPOS[b], zrows] = 1.0
    # selector matmuls (collapse r): [pair][128(U row), est_rows]
    ssel = np.zeros((N_PAIRS, 128, 64), f32)
    for ci, g in enumerate(GEO):
        pi = PAIR_OF_CHUNK[ci]
        kp0 = sum(len(GEO[c]["bands"]) for c in PAIRS[pi][:PAIRS[pi].index(ci)])
        for k, b in enumerate(g["bands"]):
            bw = BAND_WIDTH[b]
            for r in range(2):
                for s in range(2):
                    for j in range(bw):
                        urow = MBASE[ci] + g["g0off"][k] + r * 2 * bw + s * bw + j
                        erow = (s * (PAIR_EST_ROWS[pi] // 2)
                                + (kp0 + k) * bw + j)
                        ssel[pi, urow, erow] = 1.0
    # rowsel/colsel rows are in BPOS (batch-processing) order
    colsel = np.zeros((N_BANDS, N_CHUNKS), f32)
    for ci, g in enumerate(GEO):
        for b in g["bands"]:
            colsel[BPOS[b], ci] = 1.0
    ident = np.eye(128, dtype=f32)
    # pack for single contiguous DMAs: wt [C, band*128], ssel [128, pair*64]
    wt_packed = np.ascontiguousarray(wt.transpose(1, 0, 2)).reshape(C, N_BANDS * 128)
    ssel_packed = np.ascontiguousarray(ssel.transpose(1, 0, 2)).reshape(128, N_PAIRS * 64)
    return {"wt": wt_packed, "wb": wb, "wg": wg, "rowsel": rowsel,
            "colsel": colsel, "ssel": ssel_packed, "ident": ident}


def _bake_ctx(context_real, context_imag, core):
    """Per-core signed/replicated context: CTXA/CTXB [pairs, 128, T]."""
    f32 = np.float32
    ctx = np.zeros((N_PAIRS, 128, 2 * T), f32)
    ctxa = ctx[:, :, 0:T]
    ctxb = ctx[:, :, T:2 * T]
    for ci, g in enumerate(GEO):
        pi = PAIR_OF_CHUNK[ci]
        for k, b in enumerate(g["bands"]):
            bw = BAND_WIDTH[b]
            cr = context_real[b, core, :bw]         # [bw, T]
            cim = context_imag[b, core, :bw]
            r0 = MBASE[ci] + g["g0off"][k]
            cr2 = np.concatenate([cr, cr], 0)       # rows (s, j)
            ci2 = np.concatenate([cim, cim], 0)
            ctxa[pi, r0:r0 + 2 * bw] = cr2
            ctxa[pi, r0 + 2 * bw:r0 + 4 * bw] = -ci2
            ctxb[pi, r0:r0 + 2 * bw] = ci2
            ctxb[pi, r0 + 2 * bw:r0 + 4 * bw] = cr2
    return ctx


def _build_program():
    import concourse.bass as bass
    import concourse.tile as tile
    from concourse import bacc, mybir
    from contextlib import ExitStack

    f32 = mybir.dt.float32
    f32r = mybir.dt.float32r
    i32 = mybir.dt.int32
    AF = mybir.ActivationFunctionType
    ALU = mybir.AluOpType

    nc = bacc.Bacc("TRN2", target_bir_lowering=False, debug=False)

    x_dt = f32r if F32R_BANDS else f32
    s_dt = f32r if F32R_EST else f32
    x_d = nc.dram_tensor("x", [C, N_BANDS * T], x_dt, kind="ExternalInput")
    wt_d = nc.dram_tensor("wt", [C, N_BANDS * 128], x_dt, kind="ExternalInput")
    wb_d = nc.dram_tensor("wb", [128, N_CHUNKS], f32, kind="ExternalInput")
    wg_d = nc.dram_tensor("wg", [128, N_CHUNKS], f32, kind="ExternalInput")
    rsel_d = nc.dram_tensor("rowsel", [N_BANDS, 128], f32, kind="ExternalInput")
    csel_d = nc.dram_tensor("colsel", [N_BANDS, N_CHUNKS], f32, kind="ExternalInput")
    ssel_d = nc.dram_tensor("ssel", [128, N_PAIRS * 64], s_dt, kind="ExternalInput")
    id_d = nc.dram_tensor("ident", [128, 128], f32, kind="ExternalInput")
    ctx_d = nc.dram_tensor("ctx", [N_PAIRS, 128, 2 * T], f32, kind="ExternalInput")
    er_d = nc.dram_tensor("er", [N_SRC, ENC, T], f32, kind="ExternalOutput")
    ei_d = nc.dram_tensor("ei", [N_SRC, ENC, T], f32, kind="ExternalOutput")

    with tile.TileContext(nc) as tc:
        with ExitStack() as ctx:
            sb = ctx.enter_context(tc.tile_pool(name="sb", bufs=1))
            st = ctx.enter_context(tc.tile_pool(name="st", bufs=2))
            wk = ctx.enter_context(tc.tile_pool(name="wk", bufs=1))
            zp = ctx.enter_context(tc.tile_pool(name="zp", bufs=4, space="PSUM"))
            ep = ctx.enter_context(tc.tile_pool(name="ep", bufs=2, space="PSUM"))
            ep2 = ctx.enter_context(tc.tile_pool(name="ep2", bufs=2, space="PSUM"))

            # ---- batch-A input DMAs first ----
            xts = {}

            def emit_x_dmas(chunks):
                for ci in chunks:
                    g = GEO[ci]
                    nb = len(g["bands"])
                    xts[ci] = wk.tile([C, nb * T], x_dt, tag=f"x{ci}",
                                      name=f"x{ci}")
                    boff = CHUNK_BOFF[ci]
                    for k in range(nb):
                        nc.sync.dma_start(
                            xts[ci][:, k * T:(k + 1) * T],
                            x_d[:, (boff + k) * T:(boff + k + 1) * T])

            emit_x_dmas(BATCH_CHUNKS[0])

            # ---- constants ----
            identt = sb.tile([128, 128], f32, tag="ident")
            nc.sync.dma_start(identt[:], id_d[:, :])
            wbt = sb.tile([128, N_CHUNKS], f32, tag="wb")
            nc.sync.dma_start(wbt[:], wb_d[:, :])
            wgt = sb.tile([128, N_CHUNKS], f32, tag="wg")
            nc.sync.dma_start(wgt[:], wg_d[:, :])
            rsels, csels = {}, {}
            for bi in range(len(BATCH_CHUNKS)):
                nbb = len(BATCH_BANDS[bi])
                b0 = BATCH_BSTART[bi]
                rsels[bi] = sb.tile([nbb, 128], f32, tag=f"rsel{bi}",
                                    name=f"rsel{bi}")
                nc.sync.dma_start(rsels[bi][:], rsel_d[b0:b0 + nbb, :])
                csels[bi] = sb.tile([nbb, N_CHUNKS], f32, tag=f"csel{bi}",
                                    name=f"csel{bi}")
                nc.sync.dma_start(csels[bi][:], csel_d[b0:b0 + nbb, :])
            onescol = sb.tile([128, 1], f32, tag="onescol")
            nc.vector.memset(onescol[:], 1.0)
            e_sb = sb.tile([128, N_CHUNKS], f32, tag="e_sb")
            istd_sb = sb.tile([128, N_CHUNKS], f32, tag="istd_sb")
            wt_all = sb.tile([C, N_BANDS * 128], x_dt, tag="wt_all")
            for q in range(4):
                w0 = q * (N_BANDS * 128 // 4)
                w1 = (q + 1) * (N_BANDS * 128 // 4) if q < 3 else N_BANDS * 128
                nc.sync.dma_start(wt_all[:, w0:w1], wt_d[:, w0:w1])
            wts = {b: wt_all[:, b * 128:(b + 1) * 128] for b in range(N_BANDS)}
            ssel_all = sb.tile([128, N_PAIRS * 64], s_dt, tag="ssel_all")
            for q in range(2):
                s0, s1 = q * 288, (q + 1) * 288
                nc.sync.dma_start(ssel_all[:, s0:s1], ssel_d[:, s0:s1])
            ssels = {pi: ssel_all[:, pi * 64:(pi + 1) * 64]
                     for pi in range(N_PAIRS)}
            for bc in BATCH_CHUNKS[1:]:
                emit_x_dmas(bc)



            NB = N_BANDS
            glob_ps = ep.tile([1, 3 * NB], f32, tag="er_ps", name="glob_ps")

            def stats_chunk(ci):
                g = GEO[ci]
                nb = len(g["bands"])
                bp0 = BPOS[g["bands"][0]]
                xt = xts[ci]
                bnr = st.tile([128, 12 * nb], f32, tag="bnr")
                mv = st.tile([128, 2 * nb], f32, tag="mv")
                for k in range(nb):
                    for h in range(2):
                        nc.vector.bn_stats(
                            bnr[:, k * 12 + h * 6:k * 12 + h * 6 + 6],
                            xt[:, k * T + h * HALF:k * T + (h + 1) * HALF].bitcast(f32))
                    nc.vector.bn_aggr(mv[:, 2 * k:2 * k + 2],
                                      bnr[:, k * 12:k * 12 + 12])
                sq = st.tile([128, nb], f32, tag="sq")
                nc.scalar.activation(sq[:], mv[:, 0:2 * nb:2], AF.Square)
                nc.tensor.matmul(glob_ps[0:1, bp0:bp0 + nb],
                                 onescol[:], mv[:, 0:2 * nb:2])
                nc.tensor.matmul(glob_ps[0:1, NB + bp0:NB + bp0 + nb],
                                 onescol[:], mv[:, 1:2 * nb:2])
                nc.tensor.matmul(glob_ps[0:1, 2 * NB + bp0:2 * NB + bp0 + nb],
                                 onescol[:], sq[:])

            def finale(bi):
                nbb = len(BATCH_BANDS[bi])
                b0 = BATCH_BSTART[bi]
                grow = st.tile([1, 3 * nbb], f32, tag="grow", name=f"grow{bi}")
                g3 = grow[:].rearrange("p (r n) -> p r n", r=3)
                gsrc = glob_ps[0:1, :].rearrange("p (r n) -> p r n", r=3)
                nc.scalar.copy(g3, gsrc[:, :, b0:b0 + nbb])
                mu_r = st.tile([1, nbb], f32, tag="mu_r", name=f"mu_r{bi}")
                nc.vector.tensor_scalar_mul(mu_r[:], grow[0:1, 0:nbb], 1.0 / 128.0)
                var_r = st.tile([1, nbb], f32, tag="var_r", name=f"var_r{bi}")
                nc.vector.tensor_add(var_r[:], grow[0:1, nbb:2 * nbb],
                                     grow[0:1, 2 * nbb:3 * nbb])
                nc.vector.tensor_scalar_mul(var_r[:], var_r[:], 1.0 / 128.0)
                musq_r = st.tile([1, nbb], f32, tag="musq_r", name=f"musq_r{bi}")
                nc.vector.tensor_mul(musq_r[:], mu_r[:], mu_r[:])
                nc.vector.tensor_sub(var_r[:], var_r[:], musq_r[:])
                qx = st.tile([1, nbb], f32, tag="qx", name=f"qx{bi}")
                nc.vector.tensor_scalar(qx[:].bitcast(i32), var_r[:].bitcast(i32),
                                        1, None, op0=ALU.logical_shift_right)
                nc.vector.tensor_scalar(qx[:].bitcast(i32), qx[:].bitcast(i32),
                                        -1, 0x5f3759df, op0=ALU.mult, op1=ALU.add)
                qa = st.tile([1, nbb], f32, tag="qa", name=f"qa{bi}")
                istd_r = st.tile([1, nbb], f32, tag="istd_r", name=f"istd_r{bi}")
                for it in range(2):
                    nc.vector.tensor_mul(qa[:], qx[:], qx[:])
                    nc.vector.tensor_mul(qa[:], qa[:], var_r[:])
                    nc.vector.tensor_scalar(qa[:], qa[:], -0.5, 1.5,
                                            op0=ALU.mult, op1=ALU.add)
                    dst = qx[:] if it == 0 else istd_r[:]
                    nc.vector.tensor_mul(dst, qx[:], qa[:])
                alpha_r = st.tile([1, nbb], f32, tag="alpha_r", name=f"al_r{bi}")
                nc.vector.tensor_mul(alpha_r[:], mu_r[:], istd_r[:])
                at_ps = ep2.tile([nbb, 1], f32, tag="ei_ps", name=f"at_ps{bi}")
                nc.tensor.transpose(at_ps[:], alpha_r[:], identt[0:1, 0:1])
                acol = st.tile([nbb, 1], f32, tag="acol", name=f"acol{bi}")
                nc.scalar.copy(acol[:], at_ps[:])
                it_ps = ep2.tile([nbb, 1], f32, tag="ei_ps", name=f"it_ps{bi}")
                nc.tensor.transpose(it_ps[:], istd_r[:], identt[0:1, 0:1])
                icol = st.tile([nbb, 1], f32, tag="icol", name=f"icol{bi}")
                nc.scalar.copy(icol[:], it_ps[:])
                rhs_all = st.tile([nbb, 2 * N_CHUNKS], f32, tag="rhs_all",
                                  name=f"rhs{bi}")
                nc.vector.tensor_scalar_mul(rhs_all[:, 0:N_CHUNKS], csels[bi][:],
                                            acol[:, 0:1])
                nc.vector.tensor_scalar_mul(rhs_all[:, N_CHUNKS:2 * N_CHUNKS],
                                            csels[bi][:], icol[:, 0:1])
                bc_ps = ep2.tile([128, 2 * N_CHUNKS], f32, tag="ei_ps",
                                 name=f"bc_ps{bi}")
                nc.tensor.matmul(bc_ps[:], rsels[bi][:], rhs_all[:])
                col_ranges = [(0, N_CHUNKS)] if len(BATCH_CHUNKS) == 1 else (
                    [(8, 16)] if bi == 0 else [(0, 8), (16, 17)])
                for c0, c1 in col_ranges:
                    tmp_e = st.tile([128, c1 - c0], f32, tag="tmp_e",
                                    name=f"tmp_e{bi}_{c0}")
                    nc.vector.tensor_mul(tmp_e[:], wgt[:, c0:c1],
                                         bc_ps[:, c0:c1])
                    nc.vector.tensor_sub(e_sb[:, c0:c1], wbt[:, c0:c1], tmp_e[:])
                    nc.scalar.copy(istd_sb[:, c0:c1],
                                   bc_ps[:, N_CHUNKS + c0:N_CHUNKS + c1])

            m_tiles = {}

            def pair_stage(pi):
                bands_rows = PAIR_EST_ROWS[pi]
                mt = m_tiles[pi]
                ctx_t = st.tile([128, 2 * T], f32, tag="ctx")
                for q in range(4):
                    nc.sync.dma_start(ctx_t[32 * q:32 * (q + 1), :],
                                      ctx_d[pi, 32 * q:32 * (q + 1), :])
                er_sb = st.tile([64, T], f32, tag="er_sb", bufs=2)
                ei_sb = st.tile([64, T], f32, tag="ei_sb", bufs=2)
                for h in range(2):
                    hs = slice(h * HALF, (h + 1) * HALF)
                    ut = st.tile([128, HALF], s_dt, tag="U")
                    nc.vector.tensor_mul(ut[:], mt[:, hs], ctx_t[:, hs])
                    vt = st.tile([128, HALF], s_dt, tag="V")
                    nc.gpsimd.tensor_mul(vt[:], mt[:, hs],
                                         ctx_t[:, T + h * HALF:T + (h + 1) * HALF])
                    er_ps = ep.tile([64, HALF], f32, tag="er_ps", name=f"erp{pi}{h}")
                    nc.tensor.matmul(er_ps[:bands_rows, :],
                                     ssels[pi][:, :bands_rows], ut[:])
                    ei_ps = ep2.tile([64, HALF], f32, tag="ei_ps", name=f"eip{pi}{h}")
                    nc.tensor.matmul(ei_ps[:bands_rows, :],
                                     ssels[pi][:, :bands_rows], vt[:])
                    nc.scalar.copy(er_sb[:bands_rows, hs], er_ps[:bands_rows, :])
                    nc.vector.tensor_copy(ei_sb[:bands_rows, hs],
                                          ei_ps[:bands_rows, :])
                nbp = sum(len(GEO[c]["bands"]) for c in PAIRS[pi])
                bw = BAND_WIDTH[GEO[PAIRS[pi][0]]["bands"][0]]
                off = int(BAND_OFF[GEO[PAIRS[pi][0]]["bands"][0]])
                for dram, tile_sb in ((er_d, er_sb), (ei_d, ei_sb)):
                    dst = bass.AP(dram, off * T,
                                  [[ENC * T, 2], [bw * T, nbp],
                                   [T, bw], [1, T]])
                    nc.gpsimd.dma_start(dst, tile_sb[0:bands_rows, :])

            done_in_pair = {pi: 0 for pi in range(N_PAIRS)}

            def compute_chunk(ci, gate_on_pool):
                g = GEO[ci]
                bands, nb = g["bands"], len(g["bands"])
                pi = PAIR_OF_CHUNK[ci]
                xt = xts[ci]
                yt = st.tile([128, T], f32, tag="y", bufs=2)
                for h in range(2):
                    z = zp.tile([128, HALF], f32, tag="z", bufs=4)
                    for k in range(nb):
                        nc.tensor.matmul(
                            z[:], wts[bands[k]],
                            xt[:, k * T + h * HALF:k * T + (h + 1) * HALF],
                            start=(k == 0), stop=(k == nb - 1))
                    nc.scalar.activation(yt[:, h * HALF:(h + 1) * HALF], z[:],
                                         AF.Relu, bias=e_sb[:, ci:ci + 1],
                                         scale=istd_sb[:, ci:ci + 1])
                s_t = st.tile([64, T], f32, tag="s", bufs=2)
                nc.scalar.activation(s_t[0:64, :], yt[64:128, :], AF.Sigmoid)
                if pi not in m_tiles:
                    m_tiles[pi] = st.tile([128, T], f32, tag="m", name=f"m{pi}")
                mt = m_tiles[pi]
                eng = nc.gpsimd if gate_on_pool else nc.vector
                eng.tensor_mul(mt[MBASE[ci]:MBASE[ci] + 64, :],
                               yt[0:64, :], s_t[0:64, :])
                if len(PAIRS[pi]) == 1:
                    nc.vector.memset(mt[64:128, :], 0.0)
                done_in_pair[pi] += 1
                if done_in_pair[pi] == len(PAIRS[pi]):
                    pair_stage(pi)

            if len(BATCH_CHUNKS) == 1:
                for ci in BATCH_CHUNKS[0]:
                    stats_chunk(ci)
                finale(0)
                for ci in BATCH_CHUNKS[0]:
                    compute_chunk(ci, gate_on_pool=False)
            else:
                a_chunks, b_chunks = BATCH_CHUNKS
                for ci in a_chunks:
                    stats_chunk(ci)
                finale(0)
                # interleave batch-A compute with batch-B stats so the
                # in-order engine streams can overlap them
                na, nbch = len(a_chunks), len(b_chunks)
                ia = ib = 0
                while ia < na or ib < nbch:
                    if ia < na:
                        compute_chunk(a_chunks[ia], gate_on_pool=False)
                        ia += 1
                    if ib < nbch:
                        stats_chunk(b_chunks[ib])
                        ib += 1
                    if ib < nbch:
                        stats_chunk(b_chunks[ib])
                        ib += 1
                finale(1)
                for ci in b_chunks:
                    compute_chunk(ci, gate_on_pool=False)

    nc.compile()
    return nc


def _get_program():
    global _PROGRAM
    if _PROGRAM is None:
        _PROGRAM = _build_program()
    return _PROGRAM


def _run(inputs, trace=False):
    from concourse.bass_utils import run_bass_kernel_spmd
    sep = np.ascontiguousarray(np.asarray(inputs["sep_output"], np.float32))
    ctx_r = np.asarray(inputs["context_real"], np.float32)
    ctx_i = np.asarray(inputs["context_imag"], np.float32)
    gamma = np.asarray(inputs["gln_gamma"], np.float32)
    beta = np.asarray(inputs["gln_beta"], np.float32)
    conv_w = np.asarray(inputs["conv_w"], np.float32)
    conv_b = np.asarray(inputs["conv_b"], np.float32)

    global _CONSTS
    if _CONSTS is None:
        _CONSTS = _bake_consts(conv_w, conv_b, gamma, beta)
    consts = _CONSTS
    nc = _get_program()

    in_maps = []
    for core in range(B):
        x = np.ascontiguousarray(
            np.transpose(sep[core], (0, 2, 1))).reshape(C, N_BANDS * T)
        ctx = _bake_ctx(ctx_r, ctx_i, core)
        in_maps.append({
            "x": x, "ctx": ctx,
            "wt": consts["wt"], "wb": consts["wb"], "wg": consts["wg"],
            "rowsel": consts["rowsel"], "colsel": consts["colsel"],
            "ssel": consts["ssel"], "ident": consts["ident"],
        })
    res = run_bass_kernel_spmd(nc, in_maps, core_ids=list(range(B)),
                               trace=trace)
    out = np.empty((B, N_SRC, ENC, T), np.complex64)
    for core in range(B):
        out.real[core] = res.results[core]["er"]
        out.imag[core] = res.results[core]["ei"]
    return out, res


def kernel(**inputs) -> np.ndarray:
    out, _ = _run(inputs, trace=False)
    return out

